# revision 15
# baseline (speedup 1.0000x reference)
"""COOTensorProduct kernel for 8 Trainium2 NeuronCores.

Math: out[b, h] = sum_{i,j} cb[h, i*64+j] * in1[b, i] * in2[b, j]
with in1/in2 [4096, 64], cb [4096, 4096] (a Clebsch-Gordan / Wigner-3j
coupling matrix for irreps '4x0e+4x1o+4x2e+4x3o' x same -> all l3).

cb is 0.1% dense but perfectly block-structured: for each (l1, l2) pair
of irrep types the coupling is a square (2l1+1)(2l2+1) x (2l1+1)(2l2+1)
matrix (stacked l3 blocks), identical across the 4x4 multiplicity copies
(u, v). The 16 pair matrices pack block-diagonally into exactly two
128x128 stationary matrices.

Everything on the wire and in the engines is fp16 (PSUM accumulates
fp32); tolerance is 2e-2 and fp16 keeps rel err ~5e-4.
  - PE: 32 fp16 [128x128x512] matmuls at 1 cyc/row (4x faster than
    fp32's 2-pass 4 cyc/row); redundant per-matmul Ldweights are
    deduped by a BIR post-pass (only 2 weight loads remain).
  - DMA: ordered partial input DMAs on one queue (so the first (u0,v0)
    product can start ~2us earlier), 1 weight DMA, chunked output
    DMAs; fp16 halves the bytes vs fp32 and descriptor/semaphore
    count is ~4x lower than the 50-DMA fp32 baseline.
  - PSUM drained in 2-bank halves ([128,1024] fp32 -> fp16) so ACT
    and DVE copies overlap and PSUM WAR never stalls PE; DVE also
    does the 32 elementwise mults (only 2x_1p mode exists for
    TensorTensor, so 267ns processing each is its floor).

Per core (512 batch rows):
  rhs[S][u,v]  = g1[S,u] * g2[S,v]      (fp16 elementwise, DVE)
  psum[S][u,v] = W_S.T @ rhs            (fp16 matmul, fp32 PSUM)
Host does the (static, index-only) gathers/permutes and fp16<->fp32
casts; device does all FLOPs.
"""

import json
import numpy as np

# ---------------------------------------------------------------- problem
B = 4096
DIM = 64
NCORES = 8
BPC = B // NCORES  # 512 batch rows per core
LMAX = 3
NMULT = 4  # multiplicity of each l in '4x0e+4x1o+4x2e+4x3o'
LS = [l for l in range(LMAX + 1) for _ in range(NMULT)]

# block-diagonal packing of the 16 (l1,l2) pair matrices into 2 stationaries
PAIRS_A = [(3, 3), (3, 2), (2, 3), (1, 1)]
PAIRS_B = [(2, 2), (1, 3), (3, 1), (1, 2), (2, 1), (0, 3), (3, 0),
           (0, 2), (2, 0), (0, 1), (1, 0), (0, 0)]

_decomp_cache = None
_nc_cache = None


def _col_start(l, u):
    return sum((2 * ll + 1) * NMULT for ll in range(l)) + u * (2 * l + 1)


def _build_decomp():
    """Index bookkeeping only (no numerics): which cb entries form the two
    stationary matrices, which in1/in2 columns feed each partition row,
    and which output row h each psum row maps to."""
    global _decomp_cache
    if _decomp_cache is not None:
        return _decomp_cache

    # replicate build_cb_matrix's row layout
    layout = {}
    idx1 = 0
    for l1 in LS:
        idx2 = 0
        for l2 in LS:
            for l3 in range(abs(l1 - l2), l1 + l2 + 1):
                layout.setdefault(l3, []).append((l1, l2, idx1 * DIM + idx2))
            idx2 += 2 * l2 + 1
        idx1 += 2 * l1 + 1
    entry_row = {}
    row = 0
    for l3 in sorted(layout):
        for (l1, l2, co) in sorted(layout[l3], key=lambda x: x[0] * LMAX + x[1]):
            entry_row[(l3, co)] = row
            row += 2 * l3 + 1
    assert row == B

    groups = []
    for pairs in (PAIRS_A, PAIRS_B):
        assert sum((2 * a + 1) * (2 * b + 1) for a, b in pairs) == 128
        c1 = np.zeros((NMULT, 128), dtype=np.int64)
        c2 = np.zeros((NMULT, 128), dtype=np.int64)
        h_of = np.zeros((NMULT, NMULT, 128), dtype=np.int64)
        w_k, w_m, w_h, w_c = [], [], [], []  # W[k,m] = cb[h, c]
        off = 0
        for (l1, l2) in pairs:
            n1, n2 = 2 * l1 + 1, 2 * l2 + 1
            kp = n1 * n2
            kk = np.arange(kp)
            m1, m2 = kk // n2, kk % n2
            for u in range(NMULT):
                c1[u, off:off + kp] = _col_start(l1, u) + m1
            for v in range(NMULT):
                c2[v, off:off + kp] = _col_start(l2, v) + m2
            mm = 0
            for l3 in range(abs(l1 - l2), l1 + l2 + 1):
                n3 = 2 * l3 + 1
                h0 = entry_row[(l3, _col_start(l1, 0) * DIM + _col_start(l2, 0))]
                km, m3m = np.meshgrid(kk, np.arange(n3), indexing="ij")
                w_k.append((off + km).ravel())
                w_m.append((off + mm + m3m).ravel())
                w_h.append((h0 + m3m).ravel())
                w_c.append(((_col_start(l1, 0) + m1[km.ravel()]) * DIM
                            + (_col_start(l2, 0) + m2[km.ravel()])))
                for u in range(NMULT):
                    for v in range(NMULT):
                        h = entry_row[(l3, _col_start(l1, u) * DIM + _col_start(l2, v))]
                        h_of[u, v, off + mm:off + mm + n3] = np.arange(h, h + n3)
                mm += n3
            off += kp
        groups.append({
            "c1": c1, "c2": c2, "h_of": h_of,
            "w_k": np.concatenate(w_k), "w_m": np.concatenate(w_m),
            "w_h": np.concatenate(w_h), "w_c": np.concatenate(w_c),
        })

    # global output row -> h map: tile t = S*16 + u*4 + v holds rows
    # t*128 + mm  ->  h_of[S][u, v, mm]
    hglob = np.zeros(32 * 128, dtype=np.int64)
    for s, g in enumerate(groups):
        for u in range(NMULT):
            for v in range(NMULT):
                t = s * 16 + u * 4 + v
                hglob[t * 128:(t + 1) * 128] = g["h_of"][u, v]
    _decomp_cache = (groups, hglob)
    return _decomp_cache


def _split_waits(bir_bytes):
    """This container's walrus build rejects >1 sync-wait per instruction
    ("Too many sync wait commands"). Hoist extra waits onto standalone
    EventSemaphore instructions on the same engine (same lowering raw
    bass wait_ge uses)."""
    bir = json.loads(bir_bytes)
    n = 0
    for fn in bir["functions"]:
        for blk in fn["blocks"]:
            out = []
            for inst in blk["instructions"]:
                si = inst.get("sync_info")
                waits = (si or {}).get("on_wait") or []
                if len(waits) > 1:
                    for w in waits[:-1]:
                        n += 1
                        out.append({
                            "debug": inst.get("debug", 0),
                            "engine": inst["engine"],
                            "ins": [], "outs": [],
                            "name": f"I-wsplit-{n}",
                            "opcode": "EventSemaphore",
                            "sync_info": {"on_update": [], "on_wait": [w]},
                        })
                    si["on_wait"] = [waits[-1]]
                out.append(inst)
            blk["instructions"] = out
    return json.dumps(bir).encode()


def _merge_sync(a, b):
    """Merge two BIR sync_info dicts (waits and updates both concatenate)."""
    if not a:
        return b
    if not b:
        return a
    return {
        "on_wait": (a.get("on_wait") or []) + (b.get("on_wait") or []),
        "on_update": (a.get("on_update") or []) + (b.get("on_update") or []),
    }


def _dedup_ldweights(bir_bytes):
    """bacc inserts an Ldweights before every Matmult even when the
    stationary is unchanged; the PE array keeps its weights, so drop
    Ldweights whose AP matches the previous one, transplanting their
    semaphore waits/updates onto the next PE instruction."""
    bir = json.loads(bir_bytes)
    for fn in bir["functions"]:
        for blk in fn["blocks"]:
            out = []
            last_w = None
            pending = None
            for inst in blk["instructions"]:
                if inst["engine"] != "PE":
                    out.append(inst)
                    continue
                if inst["opcode"] == "Ldweights":
                    key = json.dumps(inst.get("ins"), sort_keys=True)
                    if key == last_w:
                        si = inst.get("sync_info")
                        if si and (si.get("on_wait") or si.get("on_update")):
                            pending = _merge_sync(pending, si)
                        continue  # drop the redundant load
                    last_w = key
                elif inst["opcode"] not in ("Matmult", "EventSemaphore"):
                    last_w = None  # unknown PE op: stop assuming array state
                if pending is not None:
                    inst["sync_info"] = _merge_sync(pending,
                                                    inst.get("sync_info"))
                    pending = None
                out.append(inst)
            assert pending is None, "dangling sync from dropped Ldweights"
            blk["instructions"] = out
    return json.dumps(bir).encode()


# input tile-column order inside g's j axis (per stationary group s):
#   j=0: g1[u=0], j=1..4: g2[v=0..3], j=5..7: g1[u=1..3]
# so the (u=0, v=0) product can start after the first small DMA chunk.
def _g_in0(gt, _gt, u):
    return gt[:, 0:BPC] if u == 0 else gt[:, (4 + u) * BPC:(5 + u) * BPC]


def _g_in1(gt, _gt, v):
    return gt[:, (1 + v) * BPC:(2 + v) * BPC]


# Engine facts learned from traces:
#  - GPSIMD cannot access PSUM (walrus verifier rejects).
#  - GPSIMD tensor ops on SBUF starve concurrent DVE ops of SBUF
#    bandwidth (DVE mults went 335ns -> 1546ns) — keep Pool idle.
#  - DVE mults are fast (335ns) only when both operands are slices of
#    the SAME SBUF tile; separate tiles measured 3x slower.
#  - InstTensorTensor supports only the 2x_1p DVE mode, so 267ns
#    processing per [128,512] fp16 mult is the DVE floor.
#  - A full-chunk ACT drain (1.97us) paces PSUM release when psum
#    bufs=2: drain in 2-bank halves with 4 bufs so ACT+DVE overlap.
# All 32 mults on DVE; 16 half-chunk drains: DVE takes these (chunk,
# half) pairs, ACT the rest.
DVE_COPY_HALVES = {(2, 1), (4, 1), (7, 1)}


def _build_nc():
    """Bass program, identical on all 8 cores (SPMD; per-core data differs).

    Per core: 7 input DMAs (3 chunks per stationary group + weights,
    issued from two HWDGE engines so triggers overlap), 32 DVE fp16
    mults, 32 fp16 [128x128x512] matmuls into [128, 2048] 4-bank PSUM
    chunks, 8 chunk drains PSUM->SBUF fp16 (ACT/Pool), 9 output DMAs.
    """
    global _nc_cache
    if _nc_cache is not None:
        return _nc_cache
    import concourse.bass as bass
    import concourse.mybir as mybir
    from concourse.tile import TileContext

    f32 = mybir.dt.float32
    f16 = mybir.dt.float16
    nc = bass.Bass()
    w = nc.dram_tensor("w", [128, 256], f16, kind="ExternalInput")
    g = nc.dram_tensor("g", [2, 128, 8, BPC], f16, kind="ExternalInput")
    o = nc.dram_tensor("o", [8, 128, 4, BPC], f16, kind="ExternalOutput")

    with TileContext(nc) as tc:
        with (
            tc.tile_pool(name="wpool", bufs=1) as wpool,
            tc.tile_pool(name="gpool", bufs=1) as gpool,
            tc.tile_pool(name="rhspool", bufs=12) as rhspool,
            tc.tile_pool(name="psum", bufs=2, space="PSUM") as psumpool,
            tc.tile_pool(name="opool", bufs=7) as opool,
            tc.tile_pool(name="opool2", bufs=1) as opool2,
        ):
            gts = []
            for s in range(2):
                # one SBUF tile per group (same-tile operands keep DVE
                # fast), filled by 3 ordered partial DMAs on one queue
                # so the (u0, v0) columns land first; w slots in second
                gt = gpool.tile([128, 8 * BPC], f16, tag=f"g{s}", name=f"g{s}")
                nc.sync.dma_start(out=gt[:, 0:2 * BPC], in_=g[s, :, 0:2, :])
                if s == 0:
                    wt = wpool.tile([128, 256], f16, tag="w", name="w")
                    nc.sync.dma_start(out=wt, in_=w[:, :])
                nc.sync.dma_start(out=gt[:, 2 * BPC:5 * BPC], in_=g[s, :, 2:5, :])
                nc.sync.dma_start(out=gt[:, 5 * BPC:8 * BPC], in_=g[s, :, 5:8, :])
                gts.append(gt)

            for c in range(8):  # chunk = (s, u), holds tiles v = 0..3
                s, u = c // 4, c % 4
                gt = gts[s]
                if c < 7:
                    ot = opool.tile([128, 4 * BPC], f16, tag="ot", name=f"ot{c}")
                    oth = (ot[:, 0:2 * BPC], ot[:, 2 * BPC:4 * BPC])
                else:
                    ota = opool2.tile([128, 2 * BPC], f16, tag="ot7a", name="ot7a")
                    otb = opool2.tile([128, 2 * BPC], f16, tag="ot7b", name="ot7b")
                    oth = (ota, otb)
                for h in range(2):  # half-chunk = 2 tiles = 2 PSUM banks
                    ps = psumpool.tile([128, 2 * BPC], f32, tag="ps")
                    for j in range(2):
                        v = h * 2 + j
                        rhs = rhspool.tile([128, BPC], f16, tag="rhs")
                        nc.vector.tensor_mul(
                            out=rhs, in0=_g_in0(gt, gt, u), in1=_g_in1(gt, gt, v))
                        nc.tensor.matmul(ps[:, j * BPC:(j + 1) * BPC],
                                         wt[:, s * 128:(s + 1) * 128], rhs,
                                         start=True, stop=True)
                    if (c, h) in DVE_COPY_HALVES:
                        nc.vector.tensor_copy(out=oth[h], in_=ps)
                    else:
                        nc.scalar.copy(out=oth[h], in_=ps)
                if c < 7:
                    nc.sync.dma_start(out=o[c, :, :, :], in_=ot)
                else:
                    nc.sync.dma_start(out=o[c, :, 0:2, :], in_=ota)
                    nc.sync.dma_start(out=o[c, :, 2:4, :], in_=otb)

    orig = nc.to_json_bytes
    nc.to_json_bytes = lambda: _split_waits(_dedup_ldweights(orig()))
    _nc_cache = nc
    return nc


def kernel(in1, in2, cb, _want_stats=False):
    from concourse.bass_utils import run_bass_kernel_spmd

    in1 = np.ascontiguousarray(np.asarray(in1, dtype=np.float32))
    in2 = np.ascontiguousarray(np.asarray(in2, dtype=np.float32))
    cb = np.asarray(cb, dtype=np.float32)
    groups, hglob = _build_decomp()

    # stationaries extracted straight from cb (no wigner math needed)
    wmat = np.zeros((128, 256), dtype=np.float16)
    for s, g in enumerate(groups):
        wmat[g["w_k"], s * 128 + g["w_m"]] = cb[g["w_h"], g["w_c"]]

    in_maps = []
    for c in range(NCORES):
        sl = slice(c * BPC, (c + 1) * BPC)
        b1 = in1[sl].T.astype(np.float16)  # [64, BPC]
        b2 = in2[sl].T.astype(np.float16)
        # j order: [g1 u=0, g2 v=0..3, g1 u=1..3] (see _g_in0/_g_in1)
        gg = np.empty((2, 128, 8, BPC), dtype=np.float16)
        for s, g in enumerate(groups):
            gg[s, :, 0, :] = b1[g["c1"][0]]
            for v in range(NMULT):
                gg[s, :, 1 + v, :] = b2[g["c2"][v]]
            for u in range(1, NMULT):
                gg[s, :, 4 + u, :] = b1[g["c1"][u]]
        in_maps.append({"w": wmat, "g": gg})

    nc = _build_nc()
    import os
    trace = bool(int(os.environ.get("KERNEL_TRACE", "0")))
    res = run_bass_kernel_spmd(nc, in_maps, core_ids=list(range(NCORES)),
                               trace=trace)

    # o [8, 128, 4, BPC] -> [32*128 permuted rows, B batch]
    full = np.concatenate(
        [np.asarray(r["o"]).transpose(0, 2, 1, 3).reshape(32 * 128, BPC)
         for r in res.results], axis=1)
    out = np.empty((B, B), dtype=np.float32)
    out[:, hglob] = full.T.astype(np.float32)
    if _want_stats:
        return out, res
    return out


if __name__ == "__main__":
    rng = np.random.default_rng(0)
    a = rng.standard_normal((B, DIM)).astype(np.float32)
    b = rng.standard_normal((B, DIM)).astype(np.float32)
    cb = np.load("/tmp/cb.npy")
    out = kernel(a, b, cb)
    outer = np.einsum("bi,bj->bij", a, b).reshape(B, -1)
    exp = outer @ cb.T
    print("rel err:", np.linalg.norm(out - exp) / np.linalg.norm(exp))


# revision 24
# speedup vs baseline: 1.0369x; 1.0369x over previous
"""COOTensorProduct kernel for 8 Trainium2 NeuronCores.

Math: out[b, h] = sum_{i,j} cb[h, i*64+j] * in1[b, i] * in2[b, j]
with in1/in2 [4096, 64], cb [4096, 4096] (a Clebsch-Gordan / Wigner-3j
coupling matrix for irreps '4x0e+4x1o+4x2e+4x3o' x same -> all l3).

cb is 0.1% dense but perfectly block-structured: for each (l1, l2) pair
of irrep types the coupling is a square (2l1+1)(2l2+1) x (2l1+1)(2l2+1)
matrix (stacked l3 blocks), identical across the 4x4 multiplicity copies
(u, v). The 16 pair matrices pack block-diagonally into exactly two
128x128 stationary matrices.

Everything on the wire and in the engines is fp16 (PSUM accumulates
fp32); tolerance is 2e-2 and fp16 keeps rel err ~5e-4.
  - PE: 32 fp16 [128x128x512] matmuls at 1 cyc/row (4x faster than
    fp32's 2-pass 4 cyc/row); redundant per-matmul Ldweights are
    deduped by a BIR post-pass (only 2 weight loads remain).
  - DMA: ordered partial input DMAs on one queue (so the first (u0,v0)
    product can start ~2us earlier), 1 weight DMA, chunked output
    DMAs; fp16 halves the bytes vs fp32 and descriptor/semaphore
    count is ~4x lower than the 50-DMA fp32 baseline.
  - PSUM drained in 2-bank halves ([128,1024] fp32 -> fp16) so ACT
    and DVE copies overlap and PSUM WAR never stalls PE; DVE also
    does the 32 elementwise mults (only 2x_1p mode exists for
    TensorTensor, so 267ns processing each is its floor).

Per core (512 batch rows):
  rhs[S][u,v]  = g1[S,u] * g2[S,v]      (fp16 elementwise, DVE)
  psum[S][u,v] = W_S.T @ rhs            (fp16 matmul, fp32 PSUM)
Host does the (static, index-only) gathers/permutes and fp16<->fp32
casts; device does all FLOPs.
"""

import json
import numpy as np

# ---------------------------------------------------------------- problem
B = 4096
DIM = 64
NCORES = 8
BPC = B // NCORES  # 512 batch rows per core
LMAX = 3
NMULT = 4  # multiplicity of each l in '4x0e+4x1o+4x2e+4x3o'
LS = [l for l in range(LMAX + 1) for _ in range(NMULT)]

# block-diagonal packing of the 16 (l1,l2) pair matrices into 2 stationaries
PAIRS_A = [(3, 3), (3, 2), (2, 3), (1, 1)]
PAIRS_B = [(2, 2), (1, 3), (3, 1), (1, 2), (2, 1), (0, 3), (3, 0),
           (0, 2), (2, 0), (0, 1), (1, 0), (0, 0)]

_decomp_cache = None
_nc_cache = None


def _col_start(l, u):
    return sum((2 * ll + 1) * NMULT for ll in range(l)) + u * (2 * l + 1)


def _build_decomp():
    """Index bookkeeping only (no numerics): which cb entries form the two
    stationary matrices, which in1/in2 columns feed each partition row,
    and which output row h each psum row maps to."""
    global _decomp_cache
    if _decomp_cache is not None:
        return _decomp_cache

    # replicate build_cb_matrix's row layout
    layout = {}
    idx1 = 0
    for l1 in LS:
        idx2 = 0
        for l2 in LS:
            for l3 in range(abs(l1 - l2), l1 + l2 + 1):
                layout.setdefault(l3, []).append((l1, l2, idx1 * DIM + idx2))
            idx2 += 2 * l2 + 1
        idx1 += 2 * l1 + 1
    entry_row = {}
    row = 0
    for l3 in sorted(layout):
        for (l1, l2, co) in sorted(layout[l3], key=lambda x: x[0] * LMAX + x[1]):
            entry_row[(l3, co)] = row
            row += 2 * l3 + 1
    assert row == B

    groups = []
    for pairs in (PAIRS_A, PAIRS_B):
        assert sum((2 * a + 1) * (2 * b + 1) for a, b in pairs) == 128
        c1 = np.zeros((NMULT, 128), dtype=np.int64)
        c2 = np.zeros((NMULT, 128), dtype=np.int64)
        h_of = np.zeros((NMULT, NMULT, 128), dtype=np.int64)
        w_k, w_m, w_h, w_c = [], [], [], []  # W[k,m] = cb[h, c]
        off = 0
        for (l1, l2) in pairs:
            n1, n2 = 2 * l1 + 1, 2 * l2 + 1
            kp = n1 * n2
            kk = np.arange(kp)
            m1, m2 = kk // n2, kk % n2
            for u in range(NMULT):
                c1[u, off:off + kp] = _col_start(l1, u) + m1
            for v in range(NMULT):
                c2[v, off:off + kp] = _col_start(l2, v) + m2
            mm = 0
            for l3 in range(abs(l1 - l2), l1 + l2 + 1):
                n3 = 2 * l3 + 1
                h0 = entry_row[(l3, _col_start(l1, 0) * DIM + _col_start(l2, 0))]
                km, m3m = np.meshgrid(kk, np.arange(n3), indexing="ij")
                w_k.append((off + km).ravel())
                w_m.append((off + mm + m3m).ravel())
                w_h.append((h0 + m3m).ravel())
                w_c.append(((_col_start(l1, 0) + m1[km.ravel()]) * DIM
                            + (_col_start(l2, 0) + m2[km.ravel()])))
                for u in range(NMULT):
                    for v in range(NMULT):
                        h = entry_row[(l3, _col_start(l1, u) * DIM + _col_start(l2, v))]
                        h_of[u, v, off + mm:off + mm + n3] = np.arange(h, h + n3)
                mm += n3
            off += kp
        groups.append({
            "c1": c1, "c2": c2, "h_of": h_of,
            "w_k": np.concatenate(w_k), "w_m": np.concatenate(w_m),
            "w_h": np.concatenate(w_h), "w_c": np.concatenate(w_c),
        })

    # global output row -> h map: tile t = S*16 + u*4 + v holds rows
    # t*128 + mm  ->  h_of[S][u, v, mm]
    hglob = np.zeros(32 * 128, dtype=np.int64)
    for s, g in enumerate(groups):
        for u in range(NMULT):
            for v in range(NMULT):
                t = s * 16 + u * 4 + v
                hglob[t * 128:(t + 1) * 128] = g["h_of"][u, v]
    _decomp_cache = (groups, hglob)
    return _decomp_cache


def _split_waits(bir_bytes):
    """This container's walrus build rejects >1 sync-wait per instruction
    ("Too many sync wait commands"). Hoist extra waits onto standalone
    EventSemaphore instructions on the same engine (same lowering raw
    bass wait_ge uses)."""
    bir = json.loads(bir_bytes)
    n = 0
    for fn in bir["functions"]:
        for blk in fn["blocks"]:
            out = []
            for inst in blk["instructions"]:
                si = inst.get("sync_info")
                waits = (si or {}).get("on_wait") or []
                if len(waits) > 1:
                    for w in waits[:-1]:
                        n += 1
                        out.append({
                            "debug": inst.get("debug", 0),
                            "engine": inst["engine"],
                            "ins": [], "outs": [],
                            "name": f"I-wsplit-{n}",
                            "opcode": "EventSemaphore",
                            "sync_info": {"on_update": [], "on_wait": [w]},
                        })
                    si["on_wait"] = [waits[-1]]
                out.append(inst)
            blk["instructions"] = out
    return json.dumps(bir).encode()


def _merge_sync(a, b):
    """Merge two BIR sync_info dicts (waits and updates both concatenate)."""
    if not a:
        return b
    if not b:
        return a
    return {
        "on_wait": (a.get("on_wait") or []) + (b.get("on_wait") or []),
        "on_update": (a.get("on_update") or []) + (b.get("on_update") or []),
    }


def _hoist_input_dmas(bir_bytes):
    """Move the leading wait-free SP DMA triggers (the input loads) from
    the tile-context body into the 'main' preamble block, right after
    SP's RegisterMoves and before the all-engine entry barrier.  The DMA
    rings are configured by ~5.6us while the barrier runs ~6-7.4us, so
    this starts the input transfers ~1us earlier.  Updates-only sync
    info moves with the instruction (counting semaphores only fire
    sooner, which can never deadlock a waiter)."""
    bir = json.loads(bir_bytes)
    blocks = bir["functions"][0]["blocks"]
    main, body = blocks[0], blocks[1]
    hoisted = []
    kept = []
    sp_blocked = False
    for inst in body["instructions"]:
        if inst["engine"] == "SP" and not sp_blocked:
            # SP processes each trigger for ~0.6us BEFORE reaching the
            # entry barrier, holding every engine back — so hoist only
            # the first two (g0a + w); the rest pipeline fine in-body.
            if (inst["opcode"] == "DMACopy"
                    and not ((inst.get("sync_info") or {}).get("on_wait"))
                    and len(hoisted) < 2):
                hoisted.append(inst)
                continue
            sp_blocked = True  # keep SP order past any non-hoistable inst
        kept.append(inst)
    if hoisted:
        idx = max(i for i, inst in enumerate(main["instructions"])
                  if inst["engine"] == "SP"
                  and inst["opcode"] == "RegisterMove") + 1
        main["instructions"][idx:idx] = hoisted
        body["instructions"] = kept
    return json.dumps(bir).encode()


def _dedup_ldweights(bir_bytes):
    """bacc inserts an Ldweights before every Matmult even when the
    stationary is unchanged; the PE array keeps its weights, so drop
    Ldweights whose AP matches the previous one, transplanting their
    semaphore waits/updates onto the next PE instruction."""
    bir = json.loads(bir_bytes)
    for fn in bir["functions"]:
        for blk in fn["blocks"]:
            out = []
            last_w = None
            pending = None
            for inst in blk["instructions"]:
                if inst["engine"] != "PE":
                    out.append(inst)
                    continue
                if inst["opcode"] == "Ldweights":
                    key = json.dumps(inst.get("ins"), sort_keys=True)
                    if key == last_w:
                        si = inst.get("sync_info")
                        if si and (si.get("on_wait") or si.get("on_update")):
                            pending = _merge_sync(pending, si)
                        continue  # drop the redundant load
                    last_w = key
                elif inst["opcode"] not in ("Matmult", "EventSemaphore"):
                    last_w = None  # unknown PE op: stop assuming array state
                if pending is not None:
                    inst["sync_info"] = _merge_sync(pending,
                                                    inst.get("sync_info"))
                    pending = None
                out.append(inst)
            assert pending is None, "dangling sync from dropped Ldweights"
            blk["instructions"] = out
    return json.dumps(bir).encode()


# input tile-column order inside g's j axis (per stationary group s):
#   j=0: g1[u=0], j=1..4: g2[v=0..3], j=5..7: g1[u=1..3]
# so the (u=0, v=0) product can start after the first small DMA chunk.
def _g_in0(gt, _gt, u):
    return gt[:, 0:BPC] if u == 0 else gt[:, (4 + u) * BPC:(5 + u) * BPC]


def _g_in1(gt, _gt, v):
    return gt[:, (1 + v) * BPC:(2 + v) * BPC]


# Engine facts learned from traces:
#  - GPSIMD cannot access PSUM (walrus verifier rejects).
#  - GPSIMD tensor ops on SBUF starve concurrent DVE ops of SBUF
#    bandwidth (DVE mults went 335ns -> 1546ns) — keep Pool idle.
#  - DVE mults are fast (335ns) only when both operands are slices of
#    the SAME SBUF tile; separate tiles measured 3x slower.
#  - InstTensorTensor supports only the 2x_1p DVE mode, so 267ns
#    processing per [128,512] fp16 mult is the DVE floor.
#  - A full-chunk ACT drain (1.97us) paces PSUM release when psum
#    bufs=2: drain in 2-bank halves with 4 bufs so ACT+DVE overlap.
# All 32 mults on DVE; 16 half-chunk drains: DVE takes these (chunk,
# half) pairs, ACT the rest.  DVE's copies are the LATE chunks: its
# mults front-load, so it is free at the end exactly when ACT would
# otherwise still be working its backlog down.
DVE_COPY_HALVES = {(5, 1), (6, 1), (7, 1)}


def _build_nc():
    """Bass program, identical on all 8 cores (SPMD; per-core data differs).

    Per core: 7 input DMAs (3 chunks per stationary group + weights,
    issued from two HWDGE engines so triggers overlap), 32 DVE fp16
    mults, 32 fp16 [128x128x512] matmuls into [128, 2048] 4-bank PSUM
    chunks, 8 chunk drains PSUM->SBUF fp16 (ACT/Pool), 9 output DMAs.
    """
    global _nc_cache
    if _nc_cache is not None:
        return _nc_cache
    import concourse.bass as bass
    import concourse.mybir as mybir
    from concourse.tile import TileContext

    f32 = mybir.dt.float32
    f16 = mybir.dt.float16
    nc = bass.Bass()
    w = nc.dram_tensor("w", [128, 256], f16, kind="ExternalInput")
    g = nc.dram_tensor("g", [2, 128, 8, BPC], f16, kind="ExternalInput")
    o = nc.dram_tensor("o", [8, 128, 4, BPC], f16, kind="ExternalOutput")

    with TileContext(nc) as tc:
        with (
            tc.tile_pool(name="wpool", bufs=1) as wpool,
            tc.tile_pool(name="gpool", bufs=1) as gpool,
            tc.tile_pool(name="rhspool", bufs=12) as rhspool,
            tc.tile_pool(name="psum", bufs=2, space="PSUM") as psumpool,
            tc.tile_pool(name="opool", bufs=7) as opool,
            tc.tile_pool(name="opool2", bufs=1) as opool2,
        ):
            # one SBUF tile per group (same-tile operands keep DVE
            # fast), filled by small ordered partial DMAs on one queue,
            # split at the column boundaries of first use so the mult
            # pipeline is fully fed by ~2 transfers in
            gts = []
            for s in range(2):
                gts.append(gpool.tile([128, 8 * BPC], f16, tag=f"g{s}",
                                      name=f"g{s}"))

            def g_load(s, j0, j1):
                nc.sync.dma_start(out=gts[s][:, j0 * BPC:j1 * BPC],
                                  in_=g[s, :, j0:j1, :])

            g_load(0, 0, 2)  # u0, v0
            wt = wpool.tile([128, 256], f16, tag="w", name="w")
            nc.sync.dma_start(out=wt, in_=w[:, :])
            g_load(0, 2, 4)  # v1, v2
            g_load(0, 4, 6)  # v3, u1
            g_load(0, 6, 8)  # u2, u3
            g_load(1, 0, 4)
            g_load(1, 4, 8)

            for c in range(8):  # chunk = (s, u), holds tiles v = 0..3
                s, u = c // 4, c % 4
                gt = gts[s]
                if c < 7:
                    ot = opool.tile([128, 4 * BPC], f16, tag="ot", name=f"ot{c}")
                    oth = (ot[:, 0:2 * BPC], ot[:, 2 * BPC:4 * BPC])
                else:
                    ota = opool2.tile([128, 2 * BPC], f16, tag="ot7a", name="ot7a")
                    otb = opool2.tile([128, 2 * BPC], f16, tag="ot7b", name="ot7b")
                    oth = (ota, otb)
                for h in range(2):  # half-chunk = 2 tiles = 2 PSUM banks
                    ps = psumpool.tile([128, 2 * BPC], f32, tag="ps")
                    for j in range(2):
                        v = h * 2 + j
                        rhs = rhspool.tile([128, BPC], f16, tag="rhs")
                        nc.vector.tensor_mul(
                            out=rhs, in0=_g_in0(gt, gt, u), in1=_g_in1(gt, gt, v))
                        nc.tensor.matmul(ps[:, j * BPC:(j + 1) * BPC],
                                         wt[:, s * 128:(s + 1) * 128], rhs,
                                         start=True, stop=True)
                    if (c, h) in DVE_COPY_HALVES:
                        nc.vector.tensor_copy(out=oth[h], in_=ps)
                    else:
                        nc.scalar.copy(out=oth[h], in_=ps)
                if c < 7:
                    nc.sync.dma_start(out=o[c, :, :, :], in_=ot)
                else:
                    nc.sync.dma_start(out=o[c, :, 0:2, :], in_=ota)
                    nc.sync.dma_start(out=o[c, :, 2:4, :], in_=otb)

    # NOTE: _hoist_input_dmas measured slower on HW both with 7 hoisted
    # triggers (barrier held back ~4.6us) and with 2 — left unused.
    orig = nc.to_json_bytes
    nc.to_json_bytes = lambda: _split_waits(_dedup_ldweights(orig()))
    _nc_cache = nc
    return nc


def kernel(in1, in2, cb, _want_stats=False):
    from concourse.bass_utils import run_bass_kernel_spmd

    in1 = np.ascontiguousarray(np.asarray(in1, dtype=np.float32))
    in2 = np.ascontiguousarray(np.asarray(in2, dtype=np.float32))
    cb = np.asarray(cb, dtype=np.float32)
    groups, hglob = _build_decomp()

    # stationaries extracted straight from cb (no wigner math needed)
    wmat = np.zeros((128, 256), dtype=np.float16)
    for s, g in enumerate(groups):
        wmat[g["w_k"], s * 128 + g["w_m"]] = cb[g["w_h"], g["w_c"]]

    in_maps = []
    for c in range(NCORES):
        sl = slice(c * BPC, (c + 1) * BPC)
        b1 = in1[sl].T.astype(np.float16)  # [64, BPC]
        b2 = in2[sl].T.astype(np.float16)
        # j order: [g1 u=0, g2 v=0..3, g1 u=1..3] (see _g_in0/_g_in1)
        gg = np.empty((2, 128, 8, BPC), dtype=np.float16)
        for s, g in enumerate(groups):
            gg[s, :, 0, :] = b1[g["c1"][0]]
            for v in range(NMULT):
                gg[s, :, 1 + v, :] = b2[g["c2"][v]]
            for u in range(1, NMULT):
                gg[s, :, 4 + u, :] = b1[g["c1"][u]]
        in_maps.append({"w": wmat, "g": gg})

    nc = _build_nc()
    import os
    trace = bool(int(os.environ.get("KERNEL_TRACE", "0")))
    res = run_bass_kernel_spmd(nc, in_maps, core_ids=list(range(NCORES)),
                               trace=trace)

    # o [8, 128, 4, BPC] -> [32*128 permuted rows, B batch]
    full = np.concatenate(
        [np.asarray(r["o"]).transpose(0, 2, 1, 3).reshape(32 * 128, BPC)
         for r in res.results], axis=1)
    out = np.empty((B, B), dtype=np.float32)
    out[:, hglob] = full.T.astype(np.float32)
    if _want_stats:
        return out, res
    return out


if __name__ == "__main__":
    rng = np.random.default_rng(0)
    a = rng.standard_normal((B, DIM)).astype(np.float32)
    b = rng.standard_normal((B, DIM)).astype(np.float32)
    cb = np.load("/tmp/cb.npy")
    out = kernel(a, b, cb)
    outer = np.einsum("bi,bj->bij", a, b).reshape(B, -1)
    exp = outer @ cb.T
    print("rel err:", np.linalg.norm(out - exp) / np.linalg.norm(exp))
